# revision 16
# baseline (speedup 1.0000x reference)
"""TRN2 Bass kernel for BasicLSTM (B=32, T=512, IN=512, H=1024).

Strategy: tensor-parallel over the 4H gate dim across 8 cores.
  - Core k owns gate columns [i_k | f_k | o_k | g_k], each a 128-wide
    H-slice (H indices k*128:(k+1)*128), i.e. 512 gate cols per core.
  - Phase A: xzb = x @ W_k + b_k for all (t, b) rows, stored t-major in
    DRAM scratch ([T*B, 512]).  One big GEMM, near-roofline.
  - Phase B: 512 sequential steps.  Each step:
      z   = hT.T @ U_k + xzb_t      (8 K-chunk matmuls + identity-matmul)
      i,f,o = sigmoid(z[:, :384]); g = tanh(z[:, 384:])
      c   = f*c + i*g;  h = o * tanh(c)         ([32, 128] per core)
      hT shard -> DRAM -> AllGather -> SBUF     (h^T [128, 8*32] for t+1)
  - Output: core k writes hs[:, :, k*128:(k+1)*128]; host concatenates.
"""

import numpy as np

import concourse.bass as bass
import concourse.mybir as mybir
import concourse.tile as tile
from concourse import bacc, bass_utils
from concourse.bass import ts, ds
from concourse.masks import make_identity

B = 32
T = 512
IN = 512
H = 1024
NCORES = 8
NS = 4 * H // NCORES  # 512 gate cols per core
HS = H // NCORES      # 128 h cols per core
F32 = mybir.dt.float32
AF = mybir.ActivationFunctionType


def _build(t_steps: int = T, ablate: frozenset = frozenset()):
    """ablate (perf experiments only, breaks numerics):
    'aonly' - phase A only;  'noag' - skip the AllGather (stale hT);
    'nomm'  - skip the 8 recurrent matmuls."""
    assert t_steps % 4 == 0
    nc = bacc.Bacc("TRN2", debug=False, num_devices=NCORES)

    x_d = nc.dram_tensor("x", [B, t_steps, IN], F32, kind="ExternalInput")
    w_d = nc.dram_tensor("w", [IN, NS], F32, kind="ExternalInput")
    u_d = nc.dram_tensor("u", [H, NS], F32, kind="ExternalInput")
    b_d = nc.dram_tensor("b", [1, NS], F32, kind="ExternalInput")
    hs_d = nc.dram_tensor("hs", [B, t_steps, HS], F32, kind="ExternalOutput")
    xzb_d = nc.dram_tensor("xzb", [t_steps * B, NS], F32)
    cc_in = [nc.dram_tensor(f"cc_in{i}", [HS, B], F32) for i in range(2)]
    cc_out = [
        nc.dram_tensor(f"cc_out{i}", [H, B], F32, addr_space="Shared")
        for i in range(2)
    ]
    RG = [list(range(NCORES))]

    # Long-lived constants/weights: static SBUF allocations (outside tile
    # pools, so the slot allocator can never alias them with rotating tiles).
    id128 = nc.alloc_sbuf_tensor("id128", [128, 128], F32).ap()
    id32 = nc.alloc_sbuf_tensor("id32", [32, 32], F32).ap()
    ones1 = nc.alloc_sbuf_tensor("ones1", [1, 128], F32).ap()
    b_sb = nc.alloc_sbuf_tensor("b_sb", [1, NS], F32).ap()
    wk = [nc.alloc_sbuf_tensor(f"wk{j}", [128, NS], F32).ap() for j in range(IN // 128)]
    uk = [nc.alloc_sbuf_tensor(f"uk{j}", [128, NS], F32).ap() for j in range(H // 128)]
    c_bufs = [nc.alloc_sbuf_tensor(f"c_st{i}", [B, HS], F32).ap() for i in range(2)]

    with tile.TileContext(nc) as tc:
        with (
            tc.tile_pool(name="xin", bufs=3) as xin_pool,
            tc.tile_pool(name="xtr", bufs=4) as xt_pool,
            tc.tile_pool(name="xzsb", bufs=3) as xzsb_pool,
            tc.tile_pool(name="psA", bufs=2, space=bass.MemorySpace.PSUM) as psA_pool,
            tc.tile_pool(name="psT", bufs=4, space=bass.MemorySpace.PSUM) as psT_pool,
            tc.tile_pool(name="xzt", bufs=6) as xzt_pool,
            tc.tile_pool(name="state", bufs=2) as st_pool,
            tc.tile_pool(name="gates", bufs=2) as g_pool,
            tc.tile_pool(name="hT", bufs=2) as hT_pool,
            tc.tile_pool(name="psB", bufs=2, space=bass.MemorySpace.PSUM) as psB_pool,
        ):
            nc.any.memset(c_bufs[0], 0.0)
            make_identity(nc, id128)
            make_identity(nc, id32)
            nc.any.memset(ones1, 1.0)
            nc.sync.dma_start(b_sb, b_d.ap())
            for j in range(IN // 128):
                nc.sync.dma_start(wk[j], w_d.ap()[ts(j, 128), :])
            for j in range(H // 128):
                nc.sync.dma_start(uk[j], u_d.ap()[ts(j, 128), :])

            # Phase A: xzb[t*B + b, :] = x[b, t, :] @ W_k + b_k  (t-major rows)
            ntiles = t_steps * B // 128
            for m in range(ntiles):
                t0 = m * 4
                xt_in = xin_pool.tile([128, IN], F32, tag="xin")
                nc.sync.dma_start(
                    xt_in[:, :],
                    x_d.ap()[:, ds(t0, 4), :].rearrange("b t i -> t b i"),
                )
                zp = psA_pool.tile([128, NS], F32, tag="zpa")
                for j in range(IN // 128):
                    xTp = psT_pool.tile([128, 128], F32, tag="xTp")
                    nc.tensor.transpose(xTp, xt_in[:, ts(j, 128)], id128)
                    xTs = xt_pool.tile([128, 128], F32, tag="xTs")
                    nc.vector.tensor_copy(xTs, xTp)
                    nc.tensor.matmul(zp, xTs, wk[j], start=(j == 0), stop=False)
                nc.tensor.matmul(zp, ones1, b_sb, start=False, stop=True)
                xz_sb = xzsb_pool.tile([128, NS], F32, tag="xzsb")
                nc.vector.tensor_copy(xz_sb, zp)
                nc.sync.dma_start(xzb_d.ap()[ts(m, 128), :], xz_sb)

            # Phase B: the recurrence.
            hT_cur = None  # h_0 == 0 -> step 0 skips the recurrent matmuls

            for t in range(t_steps if "aonly" not in ablate else 0):
                xzt = xzt_pool.tile([B, NS], F32, tag="xzt")
                nc.sync.dma_start(xzt, xzb_d.ap()[ts(t, B), :])

                zp = psB_pool.tile([B, NS], F32, tag="zpb")
                if t == 0 or "nomm" in ablate:
                    nc.tensor.matmul(zp, id32, xzt, start=True, stop=True)
                else:
                    for j in range(H // 128):
                        nc.tensor.matmul(
                            zp, hT_cur[:, ts(j, 32)], uk[j],
                            start=(j == 0), stop=False,
                        )
                    nc.tensor.matmul(zp, id32, xzt, start=False, stop=True)

                # gate columns: [i | f | o | g]
                sig = g_pool.tile([B, 3 * HS], F32, tag="sig")
                nc.scalar.activation(sig, zp[:, 0:3 * HS], AF.Sigmoid)
                g_t = g_pool.tile([B, HS], F32, tag="g")
                nc.scalar.activation(g_t, zp[:, 3 * HS:4 * HS], AF.Tanh)

                ig = g_pool.tile([B, HS], F32, tag="ig")
                nc.vector.tensor_mul(ig, sig[:, 0:HS], g_t)
                fc = g_pool.tile([B, HS], F32, tag="fc")
                nc.vector.tensor_mul(fc, sig[:, HS:2 * HS], c_bufs[t % 2])
                c_new = c_bufs[(t + 1) % 2]
                nc.vector.tensor_add(c_new, ig, fc)

                tc_t = g_pool.tile([B, HS], F32, tag="tc")
                nc.scalar.activation(tc_t, c_new, AF.Tanh)
                h_t = st_pool.tile([B, HS], F32, tag="h")
                nc.vector.tensor_mul(h_t, sig[:, 2 * HS:3 * HS], tc_t)

                nc.sync.dma_start(hs_d.ap()[:, ds(t, 1), :], h_t)

                if t == t_steps - 1:
                    break

                # h^T shard -> DRAM -> AllGather -> SBUF for the next step.
                htr = g_pool.tile([B, HS], F32, tag="htr")
                nc.vector.transpose(htr, h_t)
                buf = t % 2
                nc.sync.dma_start(
                    cc_in[buf].ap().rearrange("(j p) q -> p j q", j=4),
                    htr.rearrange("p (j q) -> p j q", j=4),
                )
                if "noag" in ablate:
                    hT_new = hT_pool.tile([128, H // 128 * B], F32, tag="hT")
                    nc.sync.dma_start(hT_new[:, 0:B], cc_in[buf].ap())
                    hT_cur = hT_new
                    continue
                nc.gpsimd.collective_compute(
                    "AllGather",
                    mybir.AluOpType.bypass,
                    replica_groups=RG,
                    ins=[cc_in[buf].ap().opt()],
                    outs=[cc_out[buf].ap().opt()],
                )
                hT_new = hT_pool.tile([128, H // 128 * B], F32, tag="hT")
                nc.sync.dma_start(
                    hT_new.rearrange("p (j b) -> p j b", j=H // 128),
                    cc_out[buf].ap().rearrange("(j p) b -> p j b", p=128),
                )
                hT_cur = hT_new

    nc.compile()
    return nc


def _make_in_maps(x, W, U, b, t_steps: int = T):
    x = np.asarray(x, np.float32)[:, :t_steps, :]
    W = np.asarray(W, np.float32)
    U = np.asarray(U, np.float32)
    b = np.asarray(b, np.float32)
    in_maps = []
    for k in range(NCORES):
        # per-core gate column order: [i | f | o | g]
        cols = np.concatenate(
            [np.arange(k * HS, (k + 1) * HS) + gofs * H for gofs in (0, 1, 3, 2)]
        )
        in_maps.append(
            {
                "x": np.ascontiguousarray(x),
                "w": np.ascontiguousarray(W[:, cols]),
                "u": np.ascontiguousarray(U[:, cols]),
                "b": np.ascontiguousarray(b[cols]).reshape(1, NS),
            }
        )
    return in_maps


def _pjrt_bundle(nc):
    """Reusable sharded PJRT executable (mirrors bass2jax.run_bass_via_pjrt's
    multi-core branch, but keeps the jitted callable for repeated runs)."""
    import jax
    from jax.experimental.shard_map import shard_map
    from jax.sharding import Mesh, PartitionSpec
    from concourse import bass2jax

    bass2jax.install_neuronx_cc_hook()
    partition_name = nc.partition_id_tensor.name if nc.partition_id_tensor else None
    in_names, out_names, out_avals, zero_outs = [], [], [], []
    for alloc in nc.m.functions[0].allocations:
        if not isinstance(alloc, mybir.MemoryLocationSet):
            continue
        name = alloc.memorylocations[0].name
        if alloc.kind == "ExternalInput":
            if name != partition_name:
                in_names.append(name)
        elif alloc.kind == "ExternalOutput":
            shape = tuple(alloc.tensor_shape)
            dtype = mybir.dt.np(alloc.dtype)
            out_names.append(name)
            out_avals.append(jax.core.ShapedArray(shape, dtype))
            zero_outs.append(np.zeros(shape, dtype))
    n_params = len(in_names)
    n_outs = len(out_avals)
    all_in_names = list(in_names) + list(out_names)
    if partition_name is not None:
        all_in_names.append(partition_name)

    def _body(*args):
        operands = list(args)
        if partition_name is not None:
            operands.append(bass2jax.partition_id_tensor())
        outs = bass2jax._bass_exec_p.bind(
            *operands,
            out_avals=tuple(out_avals),
            in_names=tuple(all_in_names),
            out_names=tuple(out_names),
            lowering_input_output_aliases=(),
            sim_require_finite=True,
            sim_require_nnan=True,
            nc=nc,
        )
        return tuple(outs)

    devices = jax.devices()[:NCORES]
    mesh = Mesh(np.asarray(devices), ("core",))
    in_specs = (PartitionSpec("core"),) * (n_params + n_outs)
    out_specs = (PartitionSpec("core"),) * n_outs
    sharded = jax.jit(
        shard_map(
            _body, mesh=mesh, in_specs=in_specs, out_specs=out_specs, check_rep=False
        ),
        donate_argnums=tuple(range(n_params, n_params + n_outs)),
        keep_unused=True,
    )
    return dict(
        fn=sharded,
        mesh=mesh,
        in_names=in_names,
        out_names=out_names,
        out_avals=out_avals,
        zero_outs=zero_outs,
        n_params=n_params,
    )


def _run(inputs, t_steps: int = T, trace: bool = False):
    nc = _build(t_steps)
    in_maps = _make_in_maps(inputs["x"], inputs["W"], inputs["U"], inputs["b"], t_steps)
    res = bass_utils.run_bass_kernel_spmd(
        nc, in_maps, core_ids=list(range(NCORES)), trace=trace
    )
    out = np.empty((B, t_steps, H), np.float32)
    for k in range(NCORES):
        out[:, :, k * HS:(k + 1) * HS] = res.results[k]["hs"]
    return out, res


def kernel(**inputs) -> np.ndarray:
    out, _ = _run(inputs)
    return out
